# revision 47
# baseline (speedup 1.0000x reference)
"""Trainium2 Bass kernel for AttentionFixModel (topk_masking).

Computation (per (b,t) row):
  q_proj = queries @ W_in + b_in                       [B,T,D]
  scores = einsum('btd,btnd->btn', q_proj, patch)      [B,T,N]
  attn   = softmax(scores); top-16 hard mask; renorm
  out    = einsum('btn,btnd->btd', attn, patch) @ W_out + b_out

Sharding: data-parallel over batch. B=16 across 8 cores -> 2 batches
(32 rows) per core. Weights replicated (host-packed to f16).

Per-core strategy (all on-chip data f16 except softmax intermediates):
  - patch loaded HBM->SBUF with an fp32->f16 casting DMA (gpsimd/SWDGE),
    halving DMA bytes; ten small chunks pipeline the load. W_out loads
    after the patches (it is only needed late).
  - scores: PE transposes each row's patch tiles ([d,n] layout, f16), a
    DVE/ACT copy moves them to SBUF, then per-(row,half) accumulating
    matmuls against q_projT columns produce score columns [128n, 1].
    Score matmuls trail the transposes by one row so PE never stalls.
  - top-16 runs on raw scores (imm=-1e30) so both exps (full + masked)
    happen back-to-back on ACT; softmax max comes free from the top-8
    instruction; pm/tsum fused on a DVE scalar_tensor_tensor.
  - weighted sum on PE with native-layout patch stationary and the attn
    column as moving operand ([128d, 1] outputs); the [d, row] PSUM layout
    feeds the output projection directly.
A warm-up matmul burst ramps the PE p-state while the first DMAs land.
Emission is software-pipelined (group g scores interleaved with group g-1
epilogue) so in-order engine queues never head-of-line block.
"""
import os
import sys

for _p in ("/opt/trn_rl_repo", "/root/.axon_site/_ro/trn_rl_repo"):
    if _p not in sys.path and os.path.isdir(_p):
        sys.path.append(_p)

import numpy as np
import concourse.bass as bass
import concourse.bacc as bacc
import concourse.mybir as mybir
from concourse import masks
from concourse.tile import TileContext

F32 = mybir.dt.float32
F16 = mybir.dt.float16
Alu = mybir.AluOpType
Act = mybir.ActivationFunctionType

B, T, N, D = 16, 16, 256, 384
QDIM = 384
TOPK = 16
EPS = 1e-8
NEG = -1e30
NCORES = 8
BT = (B // NCORES) * T          # rows per core = 32
NH = N // 128                   # patch partition-halves (2)
ND = D // 128                   # d-dim 128-tiles (3)
NQ = QDIM // 128                # q-dim 128-tiles (3)
NK = NH * ND                    # patchT chunks per row (6)

# patch DMA chunk sizes (rows); groups for the topk/weighted-sum batches
CHUNKS = [2, 2, 4, 4, 4, 4, 4, 4, 2, 1, 1]
GROUPS = [16, 16]
# rows whose patchT PSUM->SBUF copy runs on ACT (others on DVE); the late
# rows alternate so the final chunk's copies run on both engines in parallel
ACT_COPY_ROWS = {r for r in range(BT) if r % 2 == 1} | {2}
WARMUP_MMS = 4

W_IN_OFF = 0
W_OUT_OFF = NQ * D              # 1152
WGT_COLS = 2 * NQ * D           # 2304


def build_kernel() -> bass.Bass:
    nc = bacc.Bacc("TRN2", target_bir_lowering=False)

    sm_d = nc.dram_tensor("smalls", [BT + 33, QDIM], F16, kind="ExternalInput")
    wgt_d = nc.dram_tensor("wgt", [128, WGT_COLS], F16, kind="ExternalInput")
    p_d = nc.dram_tensor("patch_features", [BT, N, D], F32, kind="ExternalInput")
    out_d = nc.dram_tensor("out", [BT, QDIM], F32, kind="ExternalOutput")

    # DRAM view of patches: [p=128, bt, h, d]
    p_view = p_d[:].rearrange("bt (h p) d -> p bt h d", p=128)

    with TileContext(nc) as tc:
        with (
            tc.tile_pool(name="const", bufs=1) as cpool,
            tc.tile_pool(name="wgt", bufs=1) as wpool,
            tc.tile_pool(name="patch", bufs=1) as ppool,
            tc.tile_pool(name="pT", bufs=18) as spool,
            tc.tile_pool(name="rows", bufs=2) as rpool,
            tc.tile_pool(name="ptT", bufs=4, space="PSUM") as ptpool,
            tc.tile_pool(name="psc", bufs=1, space="PSUM") as scpool,
            tc.tile_pool(name="poc", bufs=1, space="PSUM") as ocpool,
            tc.tile_pool(name="ptr", bufs=1, space="PSUM") as trpool,
            tc.tile_pool(name="pfin", bufs=1, space="PSUM") as finpool,
        ):
            # ---------- small DMAs (HWDGE); W_out is loaded LAST ----------
            smalls = wpool.tile([BT + 33, QDIM], F16, tag="smalls")
            nc.sync.dma_start(smalls[:], sm_d[:])
            wgt = wpool.tile([128, WGT_COLS], F16, tag="wgt")
            queries = smalls[:BT, :]
            b_in = smalls[32:33, :]
            b_out = smalls[64:65, :]
            w_in = [wgt[:, W_IN_OFF + j * D:W_IN_OFF + (j + 1) * D]
                    for j in range(NQ)]
            w_out = [wgt[:, W_OUT_OFF + j * QDIM:W_OUT_OFF + (j + 1) * QDIM]
                     for j in range(ND)]

            # ---------- patch cast-DMAs (SWDGE, fp32 -> f16) ----------
            ident16 = cpool.tile([128, 128], F16)
            ident32 = cpool.tile([128, 128], F32)
            ones16 = cpool.tile([BT + 33, 128], F16)
            rows = []                     # global row -> (tile, idx)
            cb = 0
            for k, sz in enumerate(CHUNKS):
                pk = ppool.tile([128, sz, NH, D], F16, tag=f"patch{k}",
                                name=f"patch{k}")
                nc.gpsimd.dma_start(pk[:], p_view[:, cb:cb + sz])
                rows += [(pk, i) for i in range(sz)]
                cb += sz
                if k == 0:
                    # constants ride behind the first prep
                    masks.make_identity(nc, ident16[:])
                    masks.make_identity(nc, ident32[:])
                    nc.vector.memset(ones16[:], 1.0)
                if k == 4:
                    # W_in rides mid-stream: only q_proj (deferred) needs it,
                    # and patches get the DMA engines first
                    nc.gpsimd.dma_start(wgt[:, :W_OUT_OFF],
                                        wgt_d[:, :W_OUT_OFF])
            # W_out generated after every patch prep: its transfer queues
            # behind all patch chunks, landing just before the final chain
            nc.gpsimd.dma_start(wgt[:, W_OUT_OFF:], wgt_d[:, W_OUT_OFF:])

            # ---------- PE p-state warm-up while DMAs land ----------
            qp_ps = finpool.tile([BT, QDIM], F32, tag="pfin")
            for i in range(WARMUP_MMS):
                nc.tensor.matmul(qp_ps[0:1, :D], ones16[0:1, 0:1],
                                 smalls[0:1, :D], start=True, stop=True)

            # ---------- q_proj = queries @ W_in + b_in (all f16) ----------
            # (emitted in pieces interleaved with the first score rows so
            # nothing stalls on the mid-stream W_in DMA)
            qtr = trpool.tile([128, NQ, BT], F16, tag="tr")
            for j in range(NQ):
                nc.tensor.transpose(qtr[:, j, :],
                                    queries[:, 128 * j:128 * (j + 1)],
                                    ident16[:BT, :BT])
            qT0 = wpool.tile([128, NQ, BT], F16, tag="qT0")
            nc.vector.tensor_copy(qT0[:], qtr[:])
            qproj = wpool.tile([BT, D], F16, tag="qproj")
            qT = wpool.tile([128, NQ, BT], F16, tag="qT")

            def emit_qproj_mms():
                for j in range(NQ):
                    nc.tensor.matmul(qp_ps[:, :D], qT0[:, j, :], w_in[j],
                                     start=(j == 0), stop=False)
                nc.tensor.matmul(qp_ps[:, :D], ones16[32:33, :BT], b_in,
                                 start=False, stop=True)
                nc.scalar.copy(qproj[:], qp_ps[:, :D])

            def emit_qT():
                # transposed q_proj: [128d, j, row] for the score matmuls
                qptr = trpool.tile([128, NQ, BT], F16, tag="tr", name="qptr")
                for j in range(NQ):
                    nc.tensor.transpose(qptr[:, j, :],
                                        qproj[:, 128 * j:128 * (j + 1)],
                                        ident16[:BT, :BT])
                nc.vector.tensor_copy(qT[:], qptr[:])

            # ---------- pipeline pieces ----------
            state = {}
            pending = []                  # rows transposed+copied, MMs not yet
            trail = [1]                   # MM trailing depth

            def ensure_scol(g):
                if g not in state:
                    state[g] = {"scol": scpool.tile([128, 16, NH], F32,
                                                    tag="pscol",
                                                    name=f"scol{g}")}
                return state[g]["scol"]

            def emit_score_mms(unit):
                pT = unit_pT.pop(unit[0])
                for m, r in enumerate(unit):
                    g, r0, nr = row_group[r]
                    scol_ps = ensure_scol(g)
                    rl = r - r0
                    for h in range(NH):
                        for j in range(ND):
                            nc.tensor.matmul(scol_ps[:, rl, h:h + 1],
                                             pT[:, m, h * ND + j, :],
                                             qT[:, j, r:r + 1],
                                             start=(j == 0), stop=(j == ND - 1))

            unit_pT = {}

            def emit_unit(unit):
                """Transpose a 1-2 row unit into one PSUM tile, copy it to
                SBUF in one op; score-MMs trail so PE never waits."""
                for r in unit:
                    ensure_scol(row_group[r][0])
                nm = len(unit)
                ptr_ps = ptpool.tile([128, nm, NK, 128], F16, tag="ptT")
                for m, r in enumerate(unit):
                    pc, i = rows[r]
                    for h in range(NH):
                        for j in range(ND):
                            nc.tensor.transpose(
                                ptr_ps[:, m, h * ND + j, :],
                                pc[:, i, h, 128 * j:128 * (j + 1)],
                                ident16[:, :])
                pT = spool.tile([128, nm, NK, 128], F16, tag="pT")
                # f32-bitcast halves the element count the copy engine sees
                if unit[0] in ACT_COPY_ROWS:
                    nc.scalar.copy(pT[:, :nm].bitcast(F32),
                                   ptr_ps[:, :nm].bitcast(F32))
                else:
                    nc.vector.tensor_copy(pT[:, :nm].bitcast(F32),
                                          ptr_ps[:, :nm].bitcast(F32))
                unit_pT[unit[0]] = pT
                pending.append(unit)
                while len(pending) > trail[0]:
                    emit_score_mms(pending.pop(0))

            def flush_rows():
                while pending:
                    emit_score_mms(pending.pop(0))

            def emit_row_range(rs):
                rs = list(rs)
                while rs:
                    if False:
                        emit_unit((rs[0], rs[1]))
                        rs = rs[2:]
                    else:
                        emit_unit((rs[0],))
                        rs = rs[1:]

            def epilogue_parts(g):
                """Top-16 + weighted sum + projection for group g, split into
                pieces for software pipelining."""
                r0, nr = groups[g]
                st = {}

                def part_a():
                    # scores to row-major [nr, 256]
                    scol = rpool.tile([128, 16, NH], F32, tag="scol")
                    nc.vector.tensor_copy(scol[:, :nr, :],
                                          state[g]["scol"][:, :nr, :])
                    tr = trpool.tile([16, N], F32, tag="tr", name=f"str{g}")
                    for h in range(NH):
                        nc.tensor.transpose(tr[:nr, 128 * h:128 * (h + 1)],
                                            scol[:, :nr, h], ident32[:, :])
                    srows = rpool.tile([16, N], F32, tag="srows")
                    if g == len(GROUPS) - 1:
                        nc.vector.tensor_copy(srows[:nr, :], tr[:nr, :])
                    else:
                        nc.scalar.copy(srows[:nr, :], tr[:nr, :])
                    st.update(srows=srows)
                    # top-16 on raw scores: two max8+match_replace rounds
                    m8a = rpool.tile([16, 8], F32, tag="m8a")
                    nc.vector.max(out=m8a[:nr, :], in_=srows[:nr, :])
                    negm = rpool.tile([16, 1], F32, tag="negm")
                    nc.vector.tensor_scalar(out=negm[:nr, :],
                                            in0=m8a[:nr, 0:1], scalar1=-1.0,
                                            scalar2=None, op0=Alu.mult)
                    st.update(m8a=m8a, negm=negm)

                def part_b():
                    # exp (with z accumulation) overlaps the DVE top-16 hunt
                    p_sb = rpool.tile([16, N], F32, tag="p")
                    zden = rpool.tile([16, 1], F32, tag="z")
                    nc.scalar.activation(out=p_sb[:nr, :],
                                         in_=st["srows"][:nr, :],
                                         func=Act.Exp, bias=st["negm"][:nr, :],
                                         scale=1.0, accum_out=zden[:nr, :])
                    w1 = rpool.tile([16, N], F32, tag="w1")
                    nc.vector.match_replace(out=w1[:nr, :],
                                            in_to_replace=st["m8a"][:nr, :],
                                            in_values=st["srows"][:nr, :],
                                            imm_value=NEG)
                    m8b = rpool.tile([16, 8], F32, tag="m8b")
                    nc.vector.max(out=m8b[:nr, :], in_=w1[:nr, :])
                    # pm = p where s >= (16th largest), else 0; tsum = sum(pm)
                    pm = rpool.tile([16, N], F32, tag="pm")
                    tsum = rpool.tile([16, 1], F32, tag="t")
                    nc.vector.scalar_tensor_tensor(
                        out=pm[:nr, :], in0=st["srows"][:nr, :],
                        scalar=m8b[:nr, 7:8], in1=p_sb[:nr, :],
                        op0=Alu.is_ge, op1=Alu.mult,
                        accum_out=tsum[:nr, :])
                    den = rpool.tile([16, 1], F32, tag="den")
                    nc.vector.tensor_scalar(out=den[:nr, :],
                                            in0=zden[:nr, :],
                                            scalar1=EPS, scalar2=tsum[:nr, :],
                                            op0=Alu.mult, op1=Alu.add)
                    winv = rpool.tile([16, 1], F32, tag="winv")
                    nc.vector.reciprocal(out=winv[:nr, :], in_=den[:nr, :])
                    # diag(winv): the weight transposes scale their columns,
                    # so the renormalization rides the transpose for free
                    diagw = rpool.tile([16, 16], F32, tag="diagw")
                    nc.vector.tensor_mul(
                        diagw[:nr, :nr], ident32[:nr, :nr],
                        winv[:nr, 0:1].broadcast_to((nr, nr)))
                    st.update(pm=pm, diagw=diagw)

                def part_c():
                    wtr = trpool.tile([128, NH, 16], F32, tag="tr",
                                      name=f"wtr{g}")
                    for h in range(NH):
                        nc.tensor.matmul(wtr[:, h, :nr],
                                         st["pm"][:nr, 128 * h:128 * (h + 1)],
                                         st["diagw"][:nr, :nr],
                                         start=True, stop=True)
                    wcol = rpool.tile([128, NH, 16], F16, tag="wcol")
                    nc.vector.tensor_copy(wcol[:, :, :nr], wtr[:, :, :nr])
                    oc_ps = ocpool.tile([128, ND, 16], F32, tag="poc")
                    fin_ps = finpool.tile([BT, QDIM], F32, tag="pfin")
                    nc.tensor.matmul(fin_ps[:nr, :], ones16[64:65, :nr], b_out,
                                     start=True, stop=False)
                    st.update(wcol=wcol, oc_ps=oc_ps, fin_ps=fin_ps)
                    for rl in range(nr):
                        pc, i = rows[r0 + rl]
                        for j in range(ND):
                            for h in range(NH):
                                nc.tensor.matmul(
                                    oc_ps[:, j, rl:rl + 1],
                                    pc[:, i, h, 128 * j:128 * (j + 1)],
                                    wcol[:, h, rl:rl + 1],
                                    start=(h == 0), stop=(h == NH - 1))

                def part_d():
                    oc16 = rpool.tile([128, ND, 16], F16, tag="oc16")
                    nc.vector.tensor_copy(oc16[:, :, :nr],
                                          st["oc_ps"][:, :, :nr])
                    fin_ps = st["fin_ps"]
                    for j in range(ND):
                        nc.tensor.matmul(fin_ps[:nr, :], oc16[:, j, :nr],
                                         w_out[j], start=False,
                                         stop=(j == ND - 1))

                def part_e():
                    fin_ps = st["fin_ps"]
                    fin_sb = rpool.tile([16, QDIM], F32, tag="fin")
                    if g == len(GROUPS) - 1:
                        nc.vector.tensor_copy(fin_sb[:nr, :], fin_ps[:nr, :])
                    else:
                        nc.scalar.copy(fin_sb[:nr, :], fin_ps[:nr, :])
                    nc.sync.dma_start(out_d[r0:r0 + nr, :], fin_sb[:nr, :])

                return [part_a, part_b, part_c, part_d, part_e]

            # ---------- group/row bookkeeping ----------
            groups = []
            row_group = {}
            r0 = 0
            for g, sz in enumerate(GROUPS):
                groups.append((r0, sz))
                for r in range(r0, r0 + sz):
                    row_group[r] = (g, r0, sz)
                r0 += sz

            # ---------- software-pipelined emission ----------
            r0, nr = groups[0]
            trail[0] = 12
            emit_row_range(range(r0, r0 + 9))
            emit_qproj_mms()
            emit_row_range(range(r0 + 9, r0 + 12))
            emit_qT()
            emit_row_range(range(r0 + 12, r0 + nr))
            for g in range(1, len(GROUPS)):
                flush_rows()        # finish previous group's trailing MMs
                r0, nr = groups[g]
                parts = epilogue_parts(g - 1)
                rlist = list(range(r0, r0 + nr))
                q = max(1, nr // 4)
                slices = [rlist[i:i + q] for i in range(0, nr, q)]
                order = [("p", 0), ("s", 0), ("p", 1), ("s", 1), ("p", 2),
                         ("s", 2), ("s", 3), ("flush", 0), ("p", 3)]
                for kind, idx in order:
                    if kind == "p" and idx < len(parts):
                        parts[idx]()
                    elif kind == "s" and idx < len(slices):
                        if g == len(GROUPS) - 1 and idx == len(slices) - 1:
                            # deep trail: all transposes before any of their
                            # matmuls, so PE is never copy-blocked
                            trail[0] = 4
                        emit_row_range(slices[idx])
                    elif kind == "flush":
                        flush_rows()
                        trail[0] = 1
                prev_e = parts[4]
            flush_rows()
            lparts = epilogue_parts(len(GROUPS) - 1)
            lparts[0]()
            prev_e()
            for part in lparts[1:]:
                part()

    if not nc.is_finalized():
        nc.finalize()
    return nc


def make_in_maps(queries, patch, W_in, b_in, W_out, b_out):
    bpc = B // NCORES
    wgt = np.zeros((128, WGT_COLS), np.float16)
    wgt[:, W_IN_OFF:W_IN_OFF + NQ * D] = (
        W_in.reshape(NQ, 128, D).transpose(1, 0, 2).reshape(128, NQ * D))
    wgt[:, W_OUT_OFF:W_OUT_OFF + ND * QDIM] = (
        W_out.reshape(ND, 128, QDIM).transpose(1, 0, 2).reshape(128, ND * QDIM))
    in_maps = []
    for c in range(NCORES):
        smalls = np.zeros((BT + 33, QDIM), np.float16)
        smalls[:BT] = queries[c * bpc:(c + 1) * bpc].reshape(BT, QDIM)
        smalls[32] = b_in[0]
        smalls[64] = b_out[0]
        in_maps.append({
            "smalls": smalls,
            "wgt": wgt,
            "patch_features": np.ascontiguousarray(
                patch[c * bpc:(c + 1) * bpc].reshape(BT, N, D)),
        })
    return in_maps


_NC_CACHE = None


def kernel(**inputs) -> np.ndarray:
    global _NC_CACHE
    from concourse.bass_utils import run_bass_kernel_spmd

    queries = np.ascontiguousarray(inputs["queries"], dtype=np.float32)
    patch = np.ascontiguousarray(inputs["patch_features"], dtype=np.float32)
    W_in = np.ascontiguousarray(inputs["W_in"], dtype=np.float32)
    b_in = np.ascontiguousarray(inputs["b_in"], dtype=np.float32).reshape(1, D)
    b_out = np.ascontiguousarray(inputs["b_out"], dtype=np.float32).reshape(1, QDIM)
    W_out = np.ascontiguousarray(inputs["W_out"], dtype=np.float32)

    if _NC_CACHE is None:
        _NC_CACHE = build_kernel()
    nc = _NC_CACHE

    in_maps = make_in_maps(queries, patch, W_in, b_in, W_out, b_out)
    res = run_bass_kernel_spmd(nc, in_maps, core_ids=list(range(NCORES)))
    bpc = B // NCORES
    outs = [res.results[c]["out"].reshape(bpc, T, QDIM) for c in range(NCORES)]
    return np.concatenate(outs, axis=0)


# revision 48
# speedup vs baseline: 1.0255x; 1.0255x over previous
"""Trainium2 Bass kernel for AttentionFixModel (topk_masking).

Computation (per (b,t) row):
  q_proj = queries @ W_in + b_in                       [B,T,D]
  scores = einsum('btd,btnd->btn', q_proj, patch)      [B,T,N]
  attn   = softmax(scores); top-16 hard mask; renorm
  out    = einsum('btn,btnd->btd', attn, patch) @ W_out + b_out

Sharding: data-parallel over batch. B=16 across 8 cores -> 2 batches
(32 rows) per core. Weights replicated (host-packed to f16).

Per-core strategy (all on-chip data f16 except softmax intermediates):
  - patch loaded HBM->SBUF with an fp32->f16 casting DMA (gpsimd/SWDGE),
    halving DMA bytes; ten small chunks pipeline the load. W_out loads
    after the patches (it is only needed late).
  - scores: PE transposes each row's patch tiles ([d,n] layout, f16), a
    DVE/ACT copy moves them to SBUF, then per-(row,half) accumulating
    matmuls against q_projT columns produce score columns [128n, 1].
    Score matmuls trail the transposes by one row so PE never stalls.
  - top-16 runs on raw scores (imm=-1e30) so both exps (full + masked)
    happen back-to-back on ACT; softmax max comes free from the top-8
    instruction; pm/tsum fused on a DVE scalar_tensor_tensor.
  - weighted sum on PE with native-layout patch stationary and the attn
    column as moving operand ([128d, 1] outputs); the [d, row] PSUM layout
    feeds the output projection directly.
A warm-up matmul burst ramps the PE p-state while the first DMAs land.
Emission is software-pipelined (group g scores interleaved with group g-1
epilogue) so in-order engine queues never head-of-line block.
"""
import os
import sys

for _p in ("/opt/trn_rl_repo", "/root/.axon_site/_ro/trn_rl_repo"):
    if _p not in sys.path and os.path.isdir(_p):
        sys.path.append(_p)

import numpy as np
import concourse.bass as bass
import concourse.bacc as bacc
import concourse.mybir as mybir
from concourse import masks
from concourse.tile import TileContext

F32 = mybir.dt.float32
F16 = mybir.dt.float16
Alu = mybir.AluOpType
Act = mybir.ActivationFunctionType

B, T, N, D = 16, 16, 256, 384
QDIM = 384
TOPK = 16
EPS = 1e-8
NEG = -1e30
NCORES = 8
BT = (B // NCORES) * T          # rows per core = 32
NH = N // 128                   # patch partition-halves (2)
ND = D // 128                   # d-dim 128-tiles (3)
NQ = QDIM // 128                # q-dim 128-tiles (3)
NK = NH * ND                    # patchT chunks per row (6)

# patch DMA chunk sizes (rows); groups for the topk/weighted-sum batches
CHUNKS = [2, 2, 4, 4, 4, 4, 4, 4, 2, 1, 1]
GROUPS = [16, 16]
# rows whose patchT PSUM->SBUF copy runs on ACT (others on DVE); the late
# rows alternate so the final chunk's copies run on both engines in parallel
ACT_COPY_ROWS = {r for r in range(BT) if r % 2 == 1} | {2}
WARMUP_MMS = 4

W_IN_OFF = 0
W_OUT_OFF = NQ * D              # 1152
WGT_COLS = 2 * NQ * D           # 2304


def build_kernel() -> bass.Bass:
    nc = bacc.Bacc("TRN2", target_bir_lowering=False)

    sm_d = nc.dram_tensor("smalls", [BT + 33, QDIM], F16, kind="ExternalInput")
    wgt_d = nc.dram_tensor("wgt", [128, WGT_COLS], F16, kind="ExternalInput")
    p_d = nc.dram_tensor("patch_features", [BT, N, D], F32, kind="ExternalInput")
    out_d = nc.dram_tensor("out", [BT, QDIM], F32, kind="ExternalOutput")

    # DRAM view of patches: [p=128, bt, h, d]
    p_view = p_d[:].rearrange("bt (h p) d -> p bt h d", p=128)

    with TileContext(nc) as tc:
        with (
            tc.tile_pool(name="const", bufs=1) as cpool,
            tc.tile_pool(name="wgt", bufs=1) as wpool,
            tc.tile_pool(name="patch", bufs=1) as ppool,
            tc.tile_pool(name="pT", bufs=18) as spool,
            tc.tile_pool(name="rows", bufs=2) as rpool,
            tc.tile_pool(name="ptT", bufs=4, space="PSUM") as ptpool,
            tc.tile_pool(name="psc", bufs=1, space="PSUM") as scpool,
            tc.tile_pool(name="poc", bufs=1, space="PSUM") as ocpool,
            tc.tile_pool(name="ptr", bufs=1, space="PSUM") as trpool,
            tc.tile_pool(name="pfin", bufs=1, space="PSUM") as finpool,
        ):
            # ---------- small DMAs (HWDGE); W_out is loaded LAST ----------
            smalls = wpool.tile([BT + 33, QDIM], F16, tag="smalls")
            nc.sync.dma_start(smalls[:], sm_d[:])
            wgt = wpool.tile([128, WGT_COLS], F16, tag="wgt")
            queries = smalls[:BT, :]
            b_in = smalls[32:33, :]
            b_out = smalls[64:65, :]
            w_in = [wgt[:, W_IN_OFF + j * D:W_IN_OFF + (j + 1) * D]
                    for j in range(NQ)]
            w_out = [wgt[:, W_OUT_OFF + j * QDIM:W_OUT_OFF + (j + 1) * QDIM]
                     for j in range(ND)]

            # ---------- patch cast-DMAs (SWDGE, fp32 -> f16) ----------
            ident16 = cpool.tile([128, 128], F16)
            ident32 = cpool.tile([128, 128], F32)
            ones16 = cpool.tile([BT + 33, 128], F16)
            rows = []                     # global row -> (tile, idx)
            cb = 0
            for k, sz in enumerate(CHUNKS):
                pk = ppool.tile([128, sz, NH, D], F16, tag=f"patch{k}",
                                name=f"patch{k}")
                nc.gpsimd.dma_start(pk[:], p_view[:, cb:cb + sz])
                rows += [(pk, i) for i in range(sz)]
                cb += sz
                if k == 0:
                    # constants ride behind the first prep
                    masks.make_identity(nc, ident16[:])
                    masks.make_identity(nc, ident32[:])
                    nc.vector.memset(ones16[:], 1.0)
                if k == 1:
                    # W_in rides mid-stream: only q_proj (deferred) needs it,
                    # and patches get the DMA engines first
                    nc.gpsimd.dma_start(wgt[:, :W_OUT_OFF],
                                        wgt_d[:, :W_OUT_OFF])
            # W_out generated after every patch prep: its transfer queues
            # behind all patch chunks, landing just before the final chain
            nc.gpsimd.dma_start(wgt[:, W_OUT_OFF:], wgt_d[:, W_OUT_OFF:])

            # ---------- PE p-state warm-up while DMAs land ----------
            qp_ps = finpool.tile([BT, QDIM], F32, tag="pfin")
            for i in range(WARMUP_MMS):
                nc.tensor.matmul(qp_ps[0:1, :D], ones16[0:1, 0:1],
                                 smalls[0:1, :D], start=True, stop=True)

            # ---------- q_proj = queries @ W_in + b_in (all f16) ----------
            # (emitted in pieces interleaved with the first score rows so
            # nothing stalls on the mid-stream W_in DMA)
            qtr = trpool.tile([128, NQ, BT], F16, tag="tr")
            for j in range(NQ):
                nc.tensor.transpose(qtr[:, j, :],
                                    queries[:, 128 * j:128 * (j + 1)],
                                    ident16[:BT, :BT])
            qT0 = wpool.tile([128, NQ, BT], F16, tag="qT0")
            nc.vector.tensor_copy(qT0[:], qtr[:])
            qproj = wpool.tile([BT, D], F16, tag="qproj")
            qT = wpool.tile([128, NQ, BT], F16, tag="qT")

            def emit_qproj_mms():
                for j in range(NQ):
                    nc.tensor.matmul(qp_ps[:, :D], qT0[:, j, :], w_in[j],
                                     start=(j == 0), stop=False)
                nc.tensor.matmul(qp_ps[:, :D], ones16[32:33, :BT], b_in,
                                 start=False, stop=True)
                nc.scalar.copy(qproj[:], qp_ps[:, :D])

            def emit_qT():
                # transposed q_proj: [128d, j, row] for the score matmuls
                qptr = trpool.tile([128, NQ, BT], F16, tag="tr", name="qptr")
                for j in range(NQ):
                    nc.tensor.transpose(qptr[:, j, :],
                                        qproj[:, 128 * j:128 * (j + 1)],
                                        ident16[:BT, :BT])
                nc.vector.tensor_copy(qT[:], qptr[:])

            # ---------- pipeline pieces ----------
            state = {}
            pending = []                  # rows transposed+copied, MMs not yet
            trail = [1]                   # MM trailing depth

            def ensure_scol(g):
                if g not in state:
                    state[g] = {"scol": scpool.tile([128, 16, NH], F32,
                                                    tag="pscol",
                                                    name=f"scol{g}")}
                return state[g]["scol"]

            def emit_score_mms(unit):
                pT = unit_pT.pop(unit[0])
                for m, r in enumerate(unit):
                    g, r0, nr = row_group[r]
                    scol_ps = ensure_scol(g)
                    rl = r - r0
                    for h in range(NH):
                        for j in range(ND):
                            nc.tensor.matmul(scol_ps[:, rl, h:h + 1],
                                             pT[:, m, h * ND + j, :],
                                             qT[:, j, r:r + 1],
                                             start=(j == 0), stop=(j == ND - 1))

            unit_pT = {}

            def emit_unit(unit):
                """Transpose a 1-2 row unit into one PSUM tile, copy it to
                SBUF in one op; score-MMs trail so PE never waits."""
                for r in unit:
                    ensure_scol(row_group[r][0])
                nm = len(unit)
                ptr_ps = ptpool.tile([128, nm, NK, 128], F16, tag="ptT")
                for m, r in enumerate(unit):
                    pc, i = rows[r]
                    for h in range(NH):
                        for j in range(ND):
                            nc.tensor.transpose(
                                ptr_ps[:, m, h * ND + j, :],
                                pc[:, i, h, 128 * j:128 * (j + 1)],
                                ident16[:, :])
                pT = spool.tile([128, nm, NK, 128], F16, tag="pT")
                # f32-bitcast halves the element count the copy engine sees
                if unit[0] in ACT_COPY_ROWS:
                    nc.scalar.copy(pT[:, :nm].bitcast(F32),
                                   ptr_ps[:, :nm].bitcast(F32))
                else:
                    nc.vector.tensor_copy(pT[:, :nm].bitcast(F32),
                                          ptr_ps[:, :nm].bitcast(F32))
                unit_pT[unit[0]] = pT
                pending.append(unit)
                while len(pending) > trail[0]:
                    emit_score_mms(pending.pop(0))

            def flush_rows():
                while pending:
                    emit_score_mms(pending.pop(0))

            def emit_row_range(rs):
                rs = list(rs)
                while rs:
                    if False:
                        emit_unit((rs[0], rs[1]))
                        rs = rs[2:]
                    else:
                        emit_unit((rs[0],))
                        rs = rs[1:]

            def epilogue_parts(g):
                """Top-16 + weighted sum + projection for group g, split into
                pieces for software pipelining."""
                r0, nr = groups[g]
                st = {}

                def part_a():
                    # scores to row-major [nr, 256]
                    scol = rpool.tile([128, 16, NH], F32, tag="scol")
                    nc.vector.tensor_copy(scol[:, :nr, :],
                                          state[g]["scol"][:, :nr, :])
                    tr = trpool.tile([16, N], F32, tag="tr", name=f"str{g}")
                    for h in range(NH):
                        nc.tensor.transpose(tr[:nr, 128 * h:128 * (h + 1)],
                                            scol[:, :nr, h], ident32[:, :])
                    srows = rpool.tile([16, N], F32, tag="srows")
                    if g == len(GROUPS) - 1:
                        nc.vector.tensor_copy(srows[:nr, :], tr[:nr, :])
                    else:
                        nc.scalar.copy(srows[:nr, :], tr[:nr, :])
                    st.update(srows=srows)
                    # top-16 on raw scores: two max8+match_replace rounds
                    m8a = rpool.tile([16, 8], F32, tag="m8a")
                    nc.vector.max(out=m8a[:nr, :], in_=srows[:nr, :])
                    negm = rpool.tile([16, 1], F32, tag="negm")
                    nc.vector.tensor_scalar(out=negm[:nr, :],
                                            in0=m8a[:nr, 0:1], scalar1=-1.0,
                                            scalar2=None, op0=Alu.mult)
                    st.update(m8a=m8a, negm=negm)

                def part_b():
                    # exp (with z accumulation) overlaps the DVE top-16 hunt
                    p_sb = rpool.tile([16, N], F32, tag="p")
                    zden = rpool.tile([16, 1], F32, tag="z")
                    nc.scalar.activation(out=p_sb[:nr, :],
                                         in_=st["srows"][:nr, :],
                                         func=Act.Exp, bias=st["negm"][:nr, :],
                                         scale=1.0, accum_out=zden[:nr, :])
                    w1 = rpool.tile([16, N], F32, tag="w1")
                    nc.vector.match_replace(out=w1[:nr, :],
                                            in_to_replace=st["m8a"][:nr, :],
                                            in_values=st["srows"][:nr, :],
                                            imm_value=NEG)
                    m8b = rpool.tile([16, 8], F32, tag="m8b")
                    nc.vector.max(out=m8b[:nr, :], in_=w1[:nr, :])
                    # pm = p where s >= (16th largest), else 0; tsum = sum(pm)
                    pm = rpool.tile([16, N], F32, tag="pm")
                    tsum = rpool.tile([16, 1], F32, tag="t")
                    nc.vector.scalar_tensor_tensor(
                        out=pm[:nr, :], in0=st["srows"][:nr, :],
                        scalar=m8b[:nr, 7:8], in1=p_sb[:nr, :],
                        op0=Alu.is_ge, op1=Alu.mult,
                        accum_out=tsum[:nr, :])
                    den = rpool.tile([16, 1], F32, tag="den")
                    nc.vector.tensor_scalar(out=den[:nr, :],
                                            in0=zden[:nr, :],
                                            scalar1=EPS, scalar2=tsum[:nr, :],
                                            op0=Alu.mult, op1=Alu.add)
                    winv = rpool.tile([16, 1], F32, tag="winv")
                    nc.vector.reciprocal(out=winv[:nr, :], in_=den[:nr, :])
                    # diag(winv): the weight transposes scale their columns,
                    # so the renormalization rides the transpose for free
                    diagw = rpool.tile([16, 16], F32, tag="diagw")
                    nc.vector.tensor_mul(
                        diagw[:nr, :nr], ident32[:nr, :nr],
                        winv[:nr, 0:1].broadcast_to((nr, nr)))
                    st.update(pm=pm, diagw=diagw)

                def part_c():
                    wtr = trpool.tile([128, NH, 16], F32, tag="tr",
                                      name=f"wtr{g}")
                    for h in range(NH):
                        nc.tensor.matmul(wtr[:, h, :nr],
                                         st["pm"][:nr, 128 * h:128 * (h + 1)],
                                         st["diagw"][:nr, :nr],
                                         start=True, stop=True)
                    wcol = rpool.tile([128, NH, 16], F16, tag="wcol")
                    nc.vector.tensor_copy(wcol[:, :, :nr], wtr[:, :, :nr])
                    oc_ps = ocpool.tile([128, ND, 16], F32, tag="poc")
                    fin_ps = finpool.tile([BT, QDIM], F32, tag="pfin")
                    nc.tensor.matmul(fin_ps[:nr, :], ones16[64:65, :nr], b_out,
                                     start=True, stop=False)
                    st.update(wcol=wcol, oc_ps=oc_ps, fin_ps=fin_ps)
                    for rl in range(nr):
                        pc, i = rows[r0 + rl]
                        for j in range(ND):
                            for h in range(NH):
                                nc.tensor.matmul(
                                    oc_ps[:, j, rl:rl + 1],
                                    pc[:, i, h, 128 * j:128 * (j + 1)],
                                    wcol[:, h, rl:rl + 1],
                                    start=(h == 0), stop=(h == NH - 1))

                def part_d():
                    oc16 = rpool.tile([128, ND, 16], F16, tag="oc16")
                    nc.vector.tensor_copy(oc16[:, :, :nr],
                                          st["oc_ps"][:, :, :nr])
                    fin_ps = st["fin_ps"]
                    for j in range(ND):
                        nc.tensor.matmul(fin_ps[:nr, :], oc16[:, j, :nr],
                                         w_out[j], start=False,
                                         stop=(j == ND - 1))

                def part_e():
                    fin_ps = st["fin_ps"]
                    fin_sb = rpool.tile([16, QDIM], F32, tag="fin")
                    if g == len(GROUPS) - 1:
                        nc.vector.tensor_copy(fin_sb[:nr, :], fin_ps[:nr, :])
                    else:
                        nc.scalar.copy(fin_sb[:nr, :], fin_ps[:nr, :])
                    nc.sync.dma_start(out_d[r0:r0 + nr, :], fin_sb[:nr, :])

                return [part_a, part_b, part_c, part_d, part_e]

            # ---------- group/row bookkeeping ----------
            groups = []
            row_group = {}
            r0 = 0
            for g, sz in enumerate(GROUPS):
                groups.append((r0, sz))
                for r in range(r0, r0 + sz):
                    row_group[r] = (g, r0, sz)
                r0 += sz

            # ---------- software-pipelined emission ----------
            r0, nr = groups[0]
            trail[0] = 12
            emit_row_range(range(r0, r0 + 9))
            emit_qproj_mms()
            emit_row_range(range(r0 + 9, r0 + 12))
            emit_qT()
            emit_row_range(range(r0 + 12, r0 + nr))
            for g in range(1, len(GROUPS)):
                flush_rows()        # finish previous group's trailing MMs
                r0, nr = groups[g]
                parts = epilogue_parts(g - 1)
                rlist = list(range(r0, r0 + nr))
                q = max(1, nr // 4)
                slices = [rlist[i:i + q] for i in range(0, nr, q)]
                order = [("p", 0), ("s", 0), ("p", 1), ("s", 1), ("p", 2),
                         ("s", 2), ("s", 3), ("flush", 0), ("p", 3)]
                for kind, idx in order:
                    if kind == "p" and idx < len(parts):
                        parts[idx]()
                    elif kind == "s" and idx < len(slices):
                        if g == len(GROUPS) - 1 and idx == len(slices) - 1:
                            # deep trail: all transposes before any of their
                            # matmuls, so PE is never copy-blocked
                            trail[0] = 4
                        emit_row_range(slices[idx])
                    elif kind == "flush":
                        flush_rows()
                        trail[0] = 1
                prev_e = parts[4]
            flush_rows()
            lparts = epilogue_parts(len(GROUPS) - 1)
            lparts[0]()
            prev_e()
            for part in lparts[1:]:
                part()

    if not nc.is_finalized():
        nc.finalize()
    return nc


def make_in_maps(queries, patch, W_in, b_in, W_out, b_out):
    bpc = B // NCORES
    wgt = np.zeros((128, WGT_COLS), np.float16)
    wgt[:, W_IN_OFF:W_IN_OFF + NQ * D] = (
        W_in.reshape(NQ, 128, D).transpose(1, 0, 2).reshape(128, NQ * D))
    wgt[:, W_OUT_OFF:W_OUT_OFF + ND * QDIM] = (
        W_out.reshape(ND, 128, QDIM).transpose(1, 0, 2).reshape(128, ND * QDIM))
    in_maps = []
    for c in range(NCORES):
        smalls = np.zeros((BT + 33, QDIM), np.float16)
        smalls[:BT] = queries[c * bpc:(c + 1) * bpc].reshape(BT, QDIM)
        smalls[32] = b_in[0]
        smalls[64] = b_out[0]
        in_maps.append({
            "smalls": smalls,
            "wgt": wgt,
            "patch_features": np.ascontiguousarray(
                patch[c * bpc:(c + 1) * bpc].reshape(BT, N, D)),
        })
    return in_maps


_NC_CACHE = None


def kernel(**inputs) -> np.ndarray:
    global _NC_CACHE
    from concourse.bass_utils import run_bass_kernel_spmd

    queries = np.ascontiguousarray(inputs["queries"], dtype=np.float32)
    patch = np.ascontiguousarray(inputs["patch_features"], dtype=np.float32)
    W_in = np.ascontiguousarray(inputs["W_in"], dtype=np.float32)
    b_in = np.ascontiguousarray(inputs["b_in"], dtype=np.float32).reshape(1, D)
    b_out = np.ascontiguousarray(inputs["b_out"], dtype=np.float32).reshape(1, QDIM)
    W_out = np.ascontiguousarray(inputs["W_out"], dtype=np.float32)

    if _NC_CACHE is None:
        _NC_CACHE = build_kernel()
    nc = _NC_CACHE

    in_maps = make_in_maps(queries, patch, W_in, b_in, W_out, b_out)
    res = run_bass_kernel_spmd(nc, in_maps, core_ids=list(range(NCORES)))
    bpc = B // NCORES
    outs = [res.results[c]["out"].reshape(bpc, T, QDIM) for c in range(NCORES)]
    return np.concatenate(outs, axis=0)


# revision 49
# speedup vs baseline: 1.0411x; 1.0153x over previous
"""Trainium2 Bass kernel for AttentionFixModel (topk_masking).

Computation (per (b,t) row):
  q_proj = queries @ W_in + b_in                       [B,T,D]
  scores = einsum('btd,btnd->btn', q_proj, patch)      [B,T,N]
  attn   = softmax(scores); top-16 hard mask; renorm
  out    = einsum('btn,btnd->btd', attn, patch) @ W_out + b_out

Sharding: data-parallel over batch. B=16 across 8 cores -> 2 batches
(32 rows) per core. Weights replicated (host-packed to f16).

Per-core strategy (all on-chip data f16 except softmax intermediates):
  - patch loaded HBM->SBUF with an fp32->f16 casting DMA (gpsimd/SWDGE),
    halving DMA bytes; ten small chunks pipeline the load. W_out loads
    after the patches (it is only needed late).
  - scores: PE transposes each row's patch tiles ([d,n] layout, f16), a
    DVE/ACT copy moves them to SBUF, then per-(row,half) accumulating
    matmuls against q_projT columns produce score columns [128n, 1].
    Score matmuls trail the transposes by one row so PE never stalls.
  - top-16 runs on raw scores (imm=-1e30) so both exps (full + masked)
    happen back-to-back on ACT; softmax max comes free from the top-8
    instruction; pm/tsum fused on a DVE scalar_tensor_tensor.
  - weighted sum on PE with native-layout patch stationary and the attn
    column as moving operand ([128d, 1] outputs); the [d, row] PSUM layout
    feeds the output projection directly.
A warm-up matmul burst ramps the PE p-state while the first DMAs land.
Emission is software-pipelined (group g scores interleaved with group g-1
epilogue) so in-order engine queues never head-of-line block.
"""
import os
import sys

for _p in ("/opt/trn_rl_repo", "/root/.axon_site/_ro/trn_rl_repo"):
    if _p not in sys.path and os.path.isdir(_p):
        sys.path.append(_p)

import numpy as np
import concourse.bass as bass
import concourse.bacc as bacc
import concourse.mybir as mybir
from concourse import masks
from concourse.tile import TileContext

F32 = mybir.dt.float32
F16 = mybir.dt.float16
Alu = mybir.AluOpType
Act = mybir.ActivationFunctionType

B, T, N, D = 16, 16, 256, 384
QDIM = 384
TOPK = 16
EPS = 1e-8
NEG = -1e30
NCORES = 8
BT = (B // NCORES) * T          # rows per core = 32
NH = N // 128                   # patch partition-halves (2)
ND = D // 128                   # d-dim 128-tiles (3)
NQ = QDIM // 128                # q-dim 128-tiles (3)
NK = NH * ND                    # patchT chunks per row (6)

# patch DMA chunk sizes (rows); groups for the topk/weighted-sum batches
CHUNKS = [2, 2, 4, 4, 4, 4, 4, 4, 2, 1, 1]
GROUPS = [16, 16]
# rows whose patchT PSUM->SBUF copy runs on ACT (others on DVE); the late
# rows alternate so the final chunk's copies run on both engines in parallel
ACT_COPY_ROWS = {r for r in range(BT) if r % 2 == 1} | {2}
WARMUP_MMS = 4

W_IN_OFF = 0
W_OUT_OFF = NQ * D              # 1152
WGT_COLS = 2 * NQ * D           # 2304


def build_kernel() -> bass.Bass:
    nc = bacc.Bacc("TRN2", target_bir_lowering=False)

    sm_d = nc.dram_tensor("smalls", [BT + 33, QDIM], F16, kind="ExternalInput")
    wgt_d = nc.dram_tensor("wgt", [128, WGT_COLS], F16, kind="ExternalInput")
    p_d = nc.dram_tensor("patch_features", [BT, N, D], F32, kind="ExternalInput")
    out_d = nc.dram_tensor("out", [BT, QDIM], F32, kind="ExternalOutput")

    # DRAM view of patches: [p=128, bt, h, d]
    p_view = p_d[:].rearrange("bt (h p) d -> p bt h d", p=128)

    with TileContext(nc) as tc:
        with (
            tc.tile_pool(name="const", bufs=1) as cpool,
            tc.tile_pool(name="wgt", bufs=1) as wpool,
            tc.tile_pool(name="patch", bufs=1) as ppool,
            tc.tile_pool(name="pT", bufs=18) as spool,
            tc.tile_pool(name="rows", bufs=2) as rpool,
            tc.tile_pool(name="ptT", bufs=4, space="PSUM") as ptpool,
            tc.tile_pool(name="psc", bufs=1, space="PSUM") as scpool,
            tc.tile_pool(name="poc", bufs=1, space="PSUM") as ocpool,
            tc.tile_pool(name="ptr", bufs=1, space="PSUM") as trpool,
            tc.tile_pool(name="pfin", bufs=1, space="PSUM") as finpool,
        ):
            # ---------- small DMAs (HWDGE); W_out is loaded LAST ----------
            smalls = wpool.tile([BT + 33, QDIM], F16, tag="smalls")
            nc.sync.dma_start(smalls[:], sm_d[:])
            wgt = wpool.tile([128, WGT_COLS], F16, tag="wgt")
            queries = smalls[:BT, :]
            b_in = smalls[32:33, :]
            b_out = smalls[64:65, :]
            w_in = [wgt[:, W_IN_OFF + j * D:W_IN_OFF + (j + 1) * D]
                    for j in range(NQ)]
            w_out = [wgt[:, W_OUT_OFF + j * QDIM:W_OUT_OFF + (j + 1) * QDIM]
                     for j in range(ND)]

            # ---------- patch cast-DMAs (SWDGE, fp32 -> f16) ----------
            ident16 = cpool.tile([128, 128], F16)
            ident32 = cpool.tile([128, 128], F32)
            ones16 = cpool.tile([BT + 33, 128], F16)
            rows = []                     # global row -> (tile, idx)
            cb = 0
            for k, sz in enumerate(CHUNKS):
                pk = ppool.tile([128, sz, NH, D], F16, tag=f"patch{k}",
                                name=f"patch{k}")
                nc.gpsimd.dma_start(pk[:], p_view[:, cb:cb + sz])
                rows += [(pk, i) for i in range(sz)]
                cb += sz
                if k == 0:
                    # constants ride behind the first prep
                    masks.make_identity(nc, ident16[:])
                    masks.make_identity(nc, ident32[:])
                    nc.vector.memset(ones16[:], 1.0)
                if k == 0:
                    nc.sync.dma_start(wgt[:, :W_OUT_OFF],
                                      wgt_d[:, :W_OUT_OFF])
            # W_out generated after every patch prep: its transfer queues
            # behind all patch chunks, landing just before the final chain
            nc.gpsimd.dma_start(wgt[:, W_OUT_OFF:], wgt_d[:, W_OUT_OFF:])

            # ---------- PE p-state warm-up while DMAs land ----------
            qp_ps = finpool.tile([BT, QDIM], F32, tag="pfin")
            for i in range(WARMUP_MMS):
                nc.tensor.matmul(qp_ps[0:1, :D], ones16[0:1, 0:1],
                                 smalls[0:1, :D], start=True, stop=True)

            # ---------- q_proj = queries @ W_in + b_in (all f16) ----------
            # (emitted in pieces interleaved with the first score rows so
            # nothing stalls on the mid-stream W_in DMA)
            qtr = trpool.tile([128, NQ, BT], F16, tag="tr")
            for j in range(NQ):
                nc.tensor.transpose(qtr[:, j, :],
                                    queries[:, 128 * j:128 * (j + 1)],
                                    ident16[:BT, :BT])
            qT0 = wpool.tile([128, NQ, BT], F16, tag="qT0")
            nc.vector.tensor_copy(qT0[:], qtr[:])
            qproj = wpool.tile([BT, D], F16, tag="qproj")
            qT = wpool.tile([128, NQ, BT], F16, tag="qT")

            def emit_qproj_mms():
                for j in range(NQ):
                    nc.tensor.matmul(qp_ps[:, :D], qT0[:, j, :], w_in[j],
                                     start=(j == 0), stop=False)
                nc.tensor.matmul(qp_ps[:, :D], ones16[32:33, :BT], b_in,
                                 start=False, stop=True)
                nc.scalar.copy(qproj[:], qp_ps[:, :D])

            def emit_qT():
                # transposed q_proj: [128d, j, row] for the score matmuls
                qptr = trpool.tile([128, NQ, BT], F16, tag="tr", name="qptr")
                for j in range(NQ):
                    nc.tensor.transpose(qptr[:, j, :],
                                        qproj[:, 128 * j:128 * (j + 1)],
                                        ident16[:BT, :BT])
                nc.vector.tensor_copy(qT[:], qptr[:])

            # ---------- pipeline pieces ----------
            state = {}
            pending = []                  # rows transposed+copied, MMs not yet
            trail = [1]                   # MM trailing depth

            def ensure_scol(g):
                if g not in state:
                    state[g] = {"scol": scpool.tile([128, 16, NH], F32,
                                                    tag="pscol",
                                                    name=f"scol{g}")}
                return state[g]["scol"]

            def emit_score_mms(unit):
                pT = unit_pT.pop(unit[0])
                for m, r in enumerate(unit):
                    g, r0, nr = row_group[r]
                    scol_ps = ensure_scol(g)
                    rl = r - r0
                    for h in range(NH):
                        for j in range(ND):
                            nc.tensor.matmul(scol_ps[:, rl, h:h + 1],
                                             pT[:, m, h * ND + j, :],
                                             qT[:, j, r:r + 1],
                                             start=(j == 0), stop=(j == ND - 1))

            unit_pT = {}

            def emit_unit(unit):
                """Transpose a 1-2 row unit into one PSUM tile, copy it to
                SBUF in one op; score-MMs trail so PE never waits."""
                for r in unit:
                    ensure_scol(row_group[r][0])
                nm = len(unit)
                ptr_ps = ptpool.tile([128, nm, NK, 128], F16, tag="ptT")
                for m, r in enumerate(unit):
                    pc, i = rows[r]
                    for h in range(NH):
                        for j in range(ND):
                            nc.tensor.transpose(
                                ptr_ps[:, m, h * ND + j, :],
                                pc[:, i, h, 128 * j:128 * (j + 1)],
                                ident16[:, :])
                pT = spool.tile([128, nm, NK, 128], F16, tag="pT")
                # f32-bitcast halves the element count the copy engine sees
                if unit[0] in ACT_COPY_ROWS:
                    nc.scalar.copy(pT[:, :nm].bitcast(F32),
                                   ptr_ps[:, :nm].bitcast(F32))
                else:
                    nc.vector.tensor_copy(pT[:, :nm].bitcast(F32),
                                          ptr_ps[:, :nm].bitcast(F32))
                unit_pT[unit[0]] = pT
                pending.append(unit)
                while len(pending) > trail[0]:
                    emit_score_mms(pending.pop(0))

            def flush_rows():
                while pending:
                    emit_score_mms(pending.pop(0))

            def emit_row_range(rs):
                rs = list(rs)
                while rs:
                    if False:
                        emit_unit((rs[0], rs[1]))
                        rs = rs[2:]
                    else:
                        emit_unit((rs[0],))
                        rs = rs[1:]

            def epilogue_parts(g):
                """Top-16 + weighted sum + projection for group g, split into
                pieces for software pipelining."""
                r0, nr = groups[g]
                st = {}

                def part_a():
                    # scores to row-major [nr, 256]
                    scol = rpool.tile([128, 16, NH], F32, tag="scol")
                    nc.vector.tensor_copy(scol[:, :nr, :],
                                          state[g]["scol"][:, :nr, :])
                    tr = trpool.tile([16, N], F32, tag="tr", name=f"str{g}")
                    for h in range(NH):
                        nc.tensor.transpose(tr[:nr, 128 * h:128 * (h + 1)],
                                            scol[:, :nr, h], ident32[:, :])
                    srows = rpool.tile([16, N], F32, tag="srows")
                    if g == len(GROUPS) - 1:
                        nc.vector.tensor_copy(srows[:nr, :], tr[:nr, :])
                    else:
                        nc.scalar.copy(srows[:nr, :], tr[:nr, :])
                    st.update(srows=srows)
                    # top-16 on raw scores: two max8+match_replace rounds
                    m8a = rpool.tile([16, 8], F32, tag="m8a")
                    nc.vector.max(out=m8a[:nr, :], in_=srows[:nr, :])
                    negm = rpool.tile([16, 1], F32, tag="negm")
                    nc.vector.tensor_scalar(out=negm[:nr, :],
                                            in0=m8a[:nr, 0:1], scalar1=-1.0,
                                            scalar2=None, op0=Alu.mult)
                    st.update(m8a=m8a, negm=negm)

                def part_b():
                    # exp (with z accumulation) overlaps the DVE top-16 hunt
                    p_sb = rpool.tile([16, N], F32, tag="p")
                    zden = rpool.tile([16, 1], F32, tag="z")
                    nc.scalar.activation(out=p_sb[:nr, :],
                                         in_=st["srows"][:nr, :],
                                         func=Act.Exp, bias=st["negm"][:nr, :],
                                         scale=1.0, accum_out=zden[:nr, :])
                    w1 = rpool.tile([16, N], F32, tag="w1")
                    nc.vector.match_replace(out=w1[:nr, :],
                                            in_to_replace=st["m8a"][:nr, :],
                                            in_values=st["srows"][:nr, :],
                                            imm_value=NEG)
                    m8b = rpool.tile([16, 8], F32, tag="m8b")
                    nc.vector.max(out=m8b[:nr, :], in_=w1[:nr, :])
                    # pm = p where s >= (16th largest), else 0; tsum = sum(pm)
                    pm = rpool.tile([16, N], F32, tag="pm")
                    tsum = rpool.tile([16, 1], F32, tag="t")
                    nc.vector.scalar_tensor_tensor(
                        out=pm[:nr, :], in0=st["srows"][:nr, :],
                        scalar=m8b[:nr, 7:8], in1=p_sb[:nr, :],
                        op0=Alu.is_ge, op1=Alu.mult,
                        accum_out=tsum[:nr, :])
                    den = rpool.tile([16, 1], F32, tag="den")
                    nc.vector.tensor_scalar(out=den[:nr, :],
                                            in0=zden[:nr, :],
                                            scalar1=EPS, scalar2=tsum[:nr, :],
                                            op0=Alu.mult, op1=Alu.add)
                    winv = rpool.tile([16, 1], F32, tag="winv")
                    nc.vector.reciprocal(out=winv[:nr, :], in_=den[:nr, :])
                    # diag(winv): the weight transposes scale their columns,
                    # so the renormalization rides the transpose for free
                    diagw = rpool.tile([16, 16], F32, tag="diagw")
                    nc.vector.tensor_mul(
                        diagw[:nr, :nr], ident32[:nr, :nr],
                        winv[:nr, 0:1].broadcast_to((nr, nr)))
                    st.update(pm=pm, diagw=diagw)

                def part_c():
                    wtr = trpool.tile([128, NH, 16], F32, tag="tr",
                                      name=f"wtr{g}")
                    for h in range(NH):
                        nc.tensor.matmul(wtr[:, h, :nr],
                                         st["pm"][:nr, 128 * h:128 * (h + 1)],
                                         st["diagw"][:nr, :nr],
                                         start=True, stop=True)
                    wcol = rpool.tile([128, NH, 16], F16, tag="wcol")
                    nc.vector.tensor_copy(wcol[:, :, :nr], wtr[:, :, :nr])
                    oc_ps = ocpool.tile([128, ND, 16], F32, tag="poc")
                    fin_ps = finpool.tile([BT, QDIM], F32, tag="pfin")
                    nc.tensor.matmul(fin_ps[:nr, :], ones16[64:65, :nr], b_out,
                                     start=True, stop=False)
                    st.update(wcol=wcol, oc_ps=oc_ps, fin_ps=fin_ps)
                    for rl in range(nr):
                        pc, i = rows[r0 + rl]
                        for j in range(ND):
                            for h in range(NH):
                                nc.tensor.matmul(
                                    oc_ps[:, j, rl:rl + 1],
                                    pc[:, i, h, 128 * j:128 * (j + 1)],
                                    wcol[:, h, rl:rl + 1],
                                    start=(h == 0), stop=(h == NH - 1))

                def part_d():
                    oc16 = rpool.tile([128, ND, 16], F16, tag="oc16")
                    nc.vector.tensor_copy(oc16[:, :, :nr],
                                          st["oc_ps"][:, :, :nr])
                    fin_ps = st["fin_ps"]
                    for j in range(ND):
                        nc.tensor.matmul(fin_ps[:nr, :], oc16[:, j, :nr],
                                         w_out[j], start=False,
                                         stop=(j == ND - 1))

                def part_e():
                    fin_ps = st["fin_ps"]
                    fin_sb = rpool.tile([16, QDIM], F32, tag="fin")
                    if g == len(GROUPS) - 1:
                        nc.vector.tensor_copy(fin_sb[:nr, :], fin_ps[:nr, :])
                    else:
                        nc.scalar.copy(fin_sb[:nr, :], fin_ps[:nr, :])
                    nc.sync.dma_start(out_d[r0:r0 + nr, :], fin_sb[:nr, :])

                return [part_a, part_b, part_c, part_d, part_e]

            # ---------- group/row bookkeeping ----------
            groups = []
            row_group = {}
            r0 = 0
            for g, sz in enumerate(GROUPS):
                groups.append((r0, sz))
                for r in range(r0, r0 + sz):
                    row_group[r] = (g, r0, sz)
                r0 += sz

            # ---------- software-pipelined emission ----------
            r0, nr = groups[0]
            trail[0] = 12
            emit_row_range(range(r0, r0 + 9))
            emit_qproj_mms()
            emit_row_range(range(r0 + 9, r0 + 12))
            emit_qT()
            emit_row_range(range(r0 + 12, r0 + nr))
            for g in range(1, len(GROUPS)):
                flush_rows()        # finish previous group's trailing MMs
                r0, nr = groups[g]
                parts = epilogue_parts(g - 1)
                rlist = list(range(r0, r0 + nr))
                q = max(1, nr // 4)
                slices = [rlist[i:i + q] for i in range(0, nr, q)]
                order = [("p", 0), ("s", 0), ("p", 1), ("s", 1), ("p", 2),
                         ("s", 2), ("s", 3), ("flush", 0), ("p", 3)]
                for kind, idx in order:
                    if kind == "p" and idx < len(parts):
                        parts[idx]()
                    elif kind == "s" and idx < len(slices):
                        if g == len(GROUPS) - 1 and idx == len(slices) - 1:
                            # deep trail: all transposes before any of their
                            # matmuls, so PE is never copy-blocked
                            trail[0] = 4
                        emit_row_range(slices[idx])
                    elif kind == "flush":
                        flush_rows()
                        trail[0] = 1
                prev_e = parts[4]
            flush_rows()
            lparts = epilogue_parts(len(GROUPS) - 1)
            lparts[0]()
            prev_e()
            for part in lparts[1:]:
                part()

    if not nc.is_finalized():
        nc.finalize()
    return nc


def make_in_maps(queries, patch, W_in, b_in, W_out, b_out):
    bpc = B // NCORES
    wgt = np.zeros((128, WGT_COLS), np.float16)
    wgt[:, W_IN_OFF:W_IN_OFF + NQ * D] = (
        W_in.reshape(NQ, 128, D).transpose(1, 0, 2).reshape(128, NQ * D))
    wgt[:, W_OUT_OFF:W_OUT_OFF + ND * QDIM] = (
        W_out.reshape(ND, 128, QDIM).transpose(1, 0, 2).reshape(128, ND * QDIM))
    in_maps = []
    for c in range(NCORES):
        smalls = np.zeros((BT + 33, QDIM), np.float16)
        smalls[:BT] = queries[c * bpc:(c + 1) * bpc].reshape(BT, QDIM)
        smalls[32] = b_in[0]
        smalls[64] = b_out[0]
        in_maps.append({
            "smalls": smalls,
            "wgt": wgt,
            "patch_features": np.ascontiguousarray(
                patch[c * bpc:(c + 1) * bpc].reshape(BT, N, D)),
        })
    return in_maps


_NC_CACHE = None


def kernel(**inputs) -> np.ndarray:
    global _NC_CACHE
    from concourse.bass_utils import run_bass_kernel_spmd

    queries = np.ascontiguousarray(inputs["queries"], dtype=np.float32)
    patch = np.ascontiguousarray(inputs["patch_features"], dtype=np.float32)
    W_in = np.ascontiguousarray(inputs["W_in"], dtype=np.float32)
    b_in = np.ascontiguousarray(inputs["b_in"], dtype=np.float32).reshape(1, D)
    b_out = np.ascontiguousarray(inputs["b_out"], dtype=np.float32).reshape(1, QDIM)
    W_out = np.ascontiguousarray(inputs["W_out"], dtype=np.float32)

    if _NC_CACHE is None:
        _NC_CACHE = build_kernel()
    nc = _NC_CACHE

    in_maps = make_in_maps(queries, patch, W_in, b_in, W_out, b_out)
    res = run_bass_kernel_spmd(nc, in_maps, core_ids=list(range(NCORES)))
    bpc = B // NCORES
    outs = [res.results[c]["out"].reshape(bpc, T, QDIM) for c in range(NCORES)]
    return np.concatenate(outs, axis=0)
